# revision 32
# baseline (speedup 1.0000x reference)
"""Trainium2 Bass kernel for a mixture-of-experts Gaussian policy network.

Network (reference, all fp32):
  h  = relu(relu(x @ Wb1 + bb1) @ Wb2 + bb2)                    [B, DH]
  e_n = relu(relu(h @ We1_n + be1_n) @ We2_n + be2_n)           per expert n
  v_n = e_n @ Wv_n + bv_n ;  k_n = e_n @ Wk_n + bk_n
  q   = Wq[tid, tid] + bq[tid]
  w_n = k_n . q   (raw, unnormalized)
  res = sum_n w_n * v_n                                          [B, DV]
  t   = relu(res @ Wt1 + bt1) ;  out = t @ Wl + bl               [B, 128]
  mean, log_std = split(out); log_std clipped [-20, 2]; std = exp(log_std)

Strategy: pure data parallelism over the batch (4096 rows -> 512 per core,
8 cores, no collectives). On device everything lives transposed
([feature_partitions, batch_free]) so no transposes are ever needed:
  outT = matmul(lhsT=W[Din,Dout] tile, rhs=inT)   (PE computes lhsT.T @ rhs)

Tower fold: w_n is a per-row SCALAR, so
  res @ Wt1 = sum_n w_n * (e2_n @ (Wv_n @ Wt1))
The host precomputes Wvt_n = Wv_n @ Wt1 (same [DH, DH] shape as Wv_n) and
bvt_n = bv_n @ Wt1, eliminating the entire tower matmul phase on device:
t = relu(sum_n w_n*(e2_n @ Wvt_n) + sum_n w_n*bvt_n + bt1).

Host also folds the task-q vector into Wk (wk_eff = Wk_n @ q, c_n = bk_n . q),
so the router weight w_n comes from a rank-1 lhsT trick: lhsT[k,m] = wk_eff[k]
for all m, which materializes w_n broadcast across all 128 partitions
directly in PSUM. The expert weighting then is one fused DVE op per tile:
  e'_n = (w_psum + c_n) * e2_n        (scalar_tensor_tensor, in place)
res accumulates over experts in SBUF fp32 (PSUM is too small to hold
[DH, B] across the expert loop). The bvt_n contribution is a 9th K=8 matmul
(lhsT = bvt stack, rhs = stack of biased w_n rows). All matmuls bf16
(fp32 accum).

Startup: the first ~10us are DMA-latency-bound while xT/wb1 land, and the
PE HAM clock-gate keeps the array at 1.2 GHz until it has been busy ~3.4us.
Dummy warmup matmuls on a memset tile fill the DMA window so the array is
already at 2.4 GHz when real work starts.
"""

import os
import numpy as np
import ml_dtypes
from contextlib import ExitStack

import concourse.bass as bass
import concourse.tile as tile
from concourse import bacc, mybir
from concourse.bass_utils import run_bass_kernel_spmd

P = 128
NCORES = 8
B = 4096
BC = B // NCORES          # 512 batch rows per core
OBS, DH, NE, DK, DV, TASKS, OUT = 512, 1024, 8, 256, 1024, 10, 128
KX = OBS // P             # 4 k-tiles for the input layer
KD = DH // P              # 8 k-tiles for hidden layers
BF = mybir.dt.bfloat16
F32 = mybir.dt.float32
RELU = mybir.ActivationFunctionType.Relu
EXP = mybir.ActivationFunctionType.Exp
IDN = mybir.ActivationFunctionType.Identity
ADD = mybir.AluOpType.add
MULT = mybir.AluOpType.mult
MAX = mybir.AluOpType.max
MIN = mybir.AluOpType.min

LOG_SIG_MIN, LOG_SIG_MAX = -20.0, 2.0

NWARM = int(os.environ.get("NWARM", "14"))  # warmup matmuls (N=512 each)


def _mm(s):  # m-tile column slice
    return slice(s * P, (s + 1) * P)


def _build_kernel(ctx, tc, io):
    nc = tc.nc
    consts = ctx.enter_context(tc.tile_pool(name="consts", bufs=1))
    wexp = ctx.enter_context(tc.tile_pool(name="wexp", bufs=2))
    eact = ctx.enter_context(tc.tile_pool(name="eact", bufs=2))
    pchain = ctx.enter_context(tc.tile_pool(name="pchain", bufs=6, space="PSUM"))
    pw = ctx.enter_context(tc.tile_pool(name="pw", bufs=2, space="PSUM"))

    # ---- persistent tiles + early DMAs ----
    # Startup is DMA-latency-bound: each queue moves ~65 GB/s early on, and
    # a dispatch costs ~0.7us on the issuing engine. Only sync/scalar/gpsimd
    # can DMA; pieces are spread round-robin in consumption order: base1's
    # m=0 chain needs xT k0..3 plus only wb1[:, k, 0:128]; later m-tiles'
    # columns follow.
    xT_sb = consts.tile([P, KX, BC], BF, tag="xT")
    wb1_sb = wexp.tile([P, KX, DH], BF, tag="w1", bufs=3)
    bb1_sb = consts.tile([P, KD], F32, tag="bb1")
    # warmup source: memset is the vector queue's FIRST instruction (vector
    # cannot DMA, so this costs nothing) so the dummy matmuls can start at
    # ~0.5us, before any DMA lands.
    wsrc_sb = consts.tile([P, BC], BF, tag="wsrc")
    nc.vector.memset(wsrc_sb[:], 0.0)
    # all-ones lhsT for the router's cross-partition-sum matmul
    wones_sb = consts.tile([P, P], BF, tag="wones")
    nc.vector.memset(wones_sb[:], 1.0)
    M0 = P          # columns gating the m=0 chain
    HB2 = BC // 2
    # xT in 5 pieces and wb1's m=0 columns first: the m=0 chain's gate is
    # ~0.2 MB per queue (~4us), not the ~0.5 MB (~11us) of the old halves
    # layout. Consumption order after that: wb1 m1-3 cols, then m4-7 cols.
    # Every piece below is CONTIGUOUS in DRAM (host pre-splits the tensors):
    # strided column-slices of a [P, k, DH] DRAM tensor cost far more
    # descriptor-write time on the issuing engine and transfer slower.
    nc.sync.dma_start(out=xT_sb[:, 0, :], in_=io["xT"][:, 0, :])
    nc.scalar.dma_start(out=xT_sb[:, 1, :], in_=io["xT"][:, 1, :])
    nc.gpsimd.dma_start(out=xT_sb[:, 2, :], in_=io["xT"][:, 2, :])
    nc.sync.dma_start(out=xT_sb[:, 3, 0:HB2], in_=io["xT"][:, 3, 0:HB2])
    nc.scalar.dma_start(out=xT_sb[:, 3, HB2:BC], in_=io["xT"][:, 3, HB2:BC])
    nc.gpsimd.dma_start(out=bb1_sb[:], in_=io["bb1"][:])
    nc.gpsimd.dma_start(out=wb1_sb[:, 0, 0:M0], in_=io["wb1a"][:, 0, :])
    nc.sync.dma_start(out=wb1_sb[:, 1, 0:M0], in_=io["wb1a"][:, 1, :])
    nc.scalar.dma_start(out=wb1_sb[:, 2, 0:M0], in_=io["wb1a"][:, 2, :])
    nc.gpsimd.dma_start(out=wb1_sb[:, 3, 0:M0], in_=io["wb1a"][:, 3, :])
    HD = DH // 2
    nc.sync.dma_start(out=wb1_sb[:, 0, M0:HD], in_=io["wb1b"][:, 0, :])
    nc.scalar.dma_start(out=wb1_sb[:, 1, M0:HD], in_=io["wb1b"][:, 1, :])
    nc.gpsimd.dma_start(out=wb1_sb[:, 2, M0:HD], in_=io["wb1b"][:, 2, :])
    nc.sync.dma_start(out=wb1_sb[:, 3, M0:HD], in_=io["wb1b"][:, 3, :])
    nc.scalar.dma_start(out=wb1_sb[:, 0, HD:DH], in_=io["wb1c"][:, 0, :])
    nc.gpsimd.dma_start(out=wb1_sb[:, 1, HD:DH], in_=io["wb1c"][:, 1, :])
    nc.sync.dma_start(out=wb1_sb[:, 2, HD:DH], in_=io["wb1c"][:, 2, :])
    nc.scalar.dma_start(out=wb1_sb[:, 3, HD:DH], in_=io["wb1c"][:, 3, :])
    # wb2 in per-m-tile column chunks (all k each) so base2's m-chains gate
    # on 0.25 MB apiece instead of the whole 2 MB.
    wb2_sb = wexp.tile([P, KD, DH], BF, tag="w2")
    bb2_sb = consts.tile([P, KD], F32, tag="bb2")
    nc.gpsimd.dma_start(out=bb2_sb[:], in_=io["bb2"][:])
    qs = [nc.sync, nc.scalar, nc.gpsimd]
    KH = KD // 2
    # first two m-chunks split in k-halves across two queues each, so the
    # first base2 chains' k=0..3 matmuls can start ~2us before the full
    # chunk lands (the base phase is DMA-starved; any earlier start also
    # keeps the HAM clock-gate from re-throttling)
    nc.sync.dma_start(out=wb2_sb[:, 0:KH, _mm(0)], in_=io["wb2m"][0][:, 0:KH, :])
    nc.scalar.dma_start(out=wb2_sb[:, KH:KD, _mm(0)], in_=io["wb2m"][0][:, KH:KD, :])
    nc.scalar.dma_start(out=wb2_sb[:, 0:KH, _mm(1)], in_=io["wb2m"][1][:, 0:KH, :])
    nc.gpsimd.dma_start(out=wb2_sb[:, KH:KD, _mm(1)], in_=io["wb2m"][1][:, KH:KD, :])
    for m in range(2, KD):
        qs[m % 3].dma_start(out=wb2_sb[:, :, _mm(m)], in_=io["wb2m"][m])
    cb_sb = consts.tile([P, NE], F32, tag="cb")
    nc.sync.dma_start(out=cb_sb[:], in_=io["cb"][:])
    bvt_sb = consts.tile([NE, DH], BF, tag="bvt")
    nc.gpsimd.dma_start(out=bvt_sb[:], in_=io["bvt"][:])

    h2_sb = consts.tile([P, KD, BC], BF, tag="h2")
    res_sb = consts.tile([P, KD, BC], F32, tag="res")
    wstk_sb = consts.tile([NE, BC], BF, tag="wstk")

    # All matmul chains draw PSUM tiles from ONE 6-deep ring (6 banks) so a
    # chain reuses a PSUM buffer 6 chains back, not 3: with 3-deep rotation
    # the 4th chain of each phase stalls ~216ns on the WAR wait for the
    # phase's first PSUM reader (activation/add), which lags ~2.5 chains at
    # phase starts. pw keeps the other 2 banks.
    def chain_tile():
        return pchain.tile([P, BC], F32, name="chain")

    # ---- PE warmup ----
    # Fill the DMA-bound startup window with dummy matmuls so the HAM
    # clock-gate releases (1.2 -> 2.4 GHz) before the first real matmul.
    for i in range(NWARM):
        wp = chain_tile()
        nc.tensor.matmul(wp[:], wsrc_sb[:, 0:P], wsrc_sb[:],
                         start=True, stop=True)

    # ---- base MLP ----
    h1_sb = eact.tile([P, KD, BC], BF, tag="e1")
    for m in range(KD):
        ps = chain_tile()
        for k in range(KX):
            nc.tensor.matmul(ps[:], wb1_sb[:, k, _mm(m)], xT_sb[:, k, :],
                             start=(k == 0), stop=(k == KX - 1))
        nc.scalar.activation(h1_sb[:, m, :], ps[:], RELU, bias=bb1_sb[:, m:m + 1])
    for m in range(KD):
        ps = chain_tile()
        for k in range(KD):
            nc.tensor.matmul(ps[:], wb2_sb[:, k, _mm(m)], h1_sb[:, k, :],
                             start=(k == 0), stop=(k == KD - 1))
        nc.scalar.activation(h2_sb[:, m, :], ps[:], RELU, bias=bb2_sb[:, m:m + 1])

    # ---- expert loop ----
    # Software-pipelined: expert n's Wvt/res phase (which depends on DVE
    # e'-mult results) is emitted after expert n+1's MLP matmuls, so the
    # PE never stalls waiting on DVE. The tower is folded into Wvt, so the
    # last expert's res tiles go straight to relu (-> t) and the head.
    bt1_sb = consts.tile([P, KD], F32, tag="bt1")
    t_holder = [None]

    def emit_v_phase(n, wv_sb, e2_sb, ms=range(KD)):
        last = (n == NE - 1)
        if last and t_holder[0] is None:
            # t lives in eact's other "e2" rotation slot (expert n-1's,
            # long dead); relu of m-tile k runs as soon as its final add
            # lands, overlapping the remaining v-phase matmuls.
            t_sb = eact.tile([P, KD, BC], BF, tag="e2")
            t_holder[0] = t_sb
        for m in ms:
            vp = chain_tile()
            for k in range(KD):
                nc.tensor.matmul(vp[:], wv_sb[:, k, _mm(m)], e2_sb[:, k, :],
                                 start=(k == 0), stop=(k == KD - 1))
            if n == 0:
                nc.vector.tensor_copy(res_sb[:, m, :], vp[:])
            elif last and m == KD - 1:
                # last m-tile: add+relu in batch quarters so the head
                # chains' k=7 operands are ready ~1us earlier (each head
                # quarter only needs its own column quarter of t).
                QB = BC // 4
                for q in range(4):
                    cq = slice(q * QB, (q + 1) * QB)
                    nc.vector.tensor_tensor(res_sb[:, m, cq], res_sb[:, m, cq],
                                            vp[:, cq], op=ADD)
                    nc.scalar.activation(t_holder[0][:, m, cq],
                                         res_sb[:, m, cq], RELU,
                                         bias=bt1_sb[:, m:m + 1])
            else:
                nc.vector.tensor_tensor(res_sb[:, m, :], res_sb[:, m, :], vp[:],
                                        op=ADD)
                if last:
                    nc.scalar.activation(t_holder[0][:, m, :], res_sb[:, m, :],
                                         RELU, bias=bt1_sb[:, m:m + 1])

    pending_v = None
    for n in range(NE):
        # weight streams spread across queues (sync/vector/gpsimd) so no
        # single ring serializes the 6.3 MB per expert; scalar is kept free
        # for activations.
        # weights in column halves (contiguous in DRAM via host pre-split) so
        # the first m-chains gate on 1 MB instead of 2 MB (matters for expert
        # 0's ramp right after base2). NO dispatches on scalar: the scalar
        # queue runs the activations, and a ~1us DMA dispatch in front of a
        # group's m0 activation delays it enough that the group's m3 chain
        # stalls on the PSUM-reuse (WAR) wait.
        w1_sb = wexp.tile([P, KD, DH], BF, tag="w1", bufs=3)
        # expert 0's we1 rides the scalar queue, which is idle after wb2 --
        # on sync it would sit behind ~1 MB and stall expert 0's e1 chains
        wq1 = nc.scalar if n == 0 else nc.sync
        wq1.dma_start(out=w1_sb[:, :, 0:HD], in_=io["we1"][n, 0])
        wq1.dma_start(out=w1_sb[:, :, HD:DH], in_=io["we1"][n, 1])
        b1_sb = wexp.tile([P, KD], F32, tag="be1")
        nc.gpsimd.dma_start(out=b1_sb[:], in_=io["be1"][n])
        wke_sb = wexp.tile([P, KD], F32, tag="wke")
        nc.gpsimd.dma_start(out=wke_sb[:], in_=io["wke"][n])
        w2_sb = wexp.tile([P, KD, DH], BF, tag="w2")
        nc.gpsimd.dma_start(out=w2_sb[:, :, 0:HD], in_=io["we2"][n, 0])
        nc.gpsimd.dma_start(out=w2_sb[:, :, HD:DH], in_=io["we2"][n, 1])
        b2_sb = wexp.tile([P, KD], F32, tag="be2")
        nc.gpsimd.dma_start(out=b2_sb[:], in_=io["be2"][n])
        wv_sb = wexp.tile([P, KD, DH], BF, tag="wv")
        nc.sync.dma_start(out=wv_sb[:, :, 0:HD], in_=io["wvt"][n, 0])
        nc.sync.dma_start(out=wv_sb[:, :, HD:DH], in_=io["wvt"][n, 1])
        if n == NE - 1:
            nc.gpsimd.dma_start(out=bt1_sb[:], in_=io["bt1"][:])

        e1_sb = eact.tile([P, KD, BC], BF, tag="e1")
        for m in range(KD):
            ps = chain_tile()
            for k in range(KD):
                nc.tensor.matmul(ps[:], w1_sb[:, k, _mm(m)], h2_sb[:, k, :],
                                 start=(k == 0), stop=(k == KD - 1))
            nc.scalar.activation(e1_sb[:, m, :], ps[:], RELU, bias=b1_sb[:, m:m + 1])

        e2_sb = eact.tile([P, KD, BC], BF, tag="e2")
        for m in range(KD):
            ps = chain_tile()
            for k in range(KD):
                nc.tensor.matmul(ps[:], w2_sb[:, k, _mm(m)], e1_sb[:, k, :],
                                 start=(k == 0), stop=(k == KD - 1))
            nc.scalar.activation(e2_sb[:, m, :], ps[:], RELU, bias=b2_sb[:, m:m + 1])

        # router weight w_n = e2_n . wk_n: the contraction is done as 8 DVE
        # multiply-accumulates (per-partition partial dots over the k-tile
        # axis; vector engine has slack) + ONE cross-partition-sum matmul
        # with an all-ones lhsT, which also materializes w broadcast on all
        # 128 partitions. This replaces the old 8-matmul rank-1 chain: -7
        # matmuls of PE time per expert (~12us total).
        racc_sb = eact.tile([P, BC], F32, tag="racc")
        raccb_sb = eact.tile([P, BC], BF, tag="raccb")
        nc.vector.tensor_scalar(out=racc_sb[:], in0=e2_sb[:, 0, :],
                                scalar1=wke_sb[:, 0:1], scalar2=None, op0=MULT)
        for k in range(1, KD - 1):
            nc.vector.scalar_tensor_tensor(out=racc_sb[:], in0=e2_sb[:, k, :],
                                           scalar=wke_sb[:, k:k + 1],
                                           in1=racc_sb[:], op0=MULT, op1=ADD)
        nc.vector.scalar_tensor_tensor(out=raccb_sb[:], in0=e2_sb[:, KD - 1, :],
                                       scalar=wke_sb[:, KD - 1:KD],
                                       in1=racc_sb[:], op0=MULT, op1=ADD)

        last = (n == NE - 1)
        # first half of the previous expert's v-phase runs while the DVE
        # dots drain, so the ones-matmul below never waits on them. For the
        # LAST expert the ones-matmul runs first (eats a ~1.6us dot wait) so
        # wsb and the e' multiplies complete while v(n-1) streams --
        # otherwise v(7)'s chains catch up with the multiplies and stall.
        if pending_v is not None and not last:
            emit_v_phase(*pending_v, ms=range(0, KD // 2))

        wps = pw.tile([P, BC], F32, name="wps")
        nc.tensor.matmul(wps[:], wones_sb[:], raccb_sb[:], start=True, stop=True)
        # biased router weight in SBUF (engine PSUM reads need 32-aligned
        # base partitions, so bias the full tile once, then DMA row 0)
        wsb = eact.tile([P, BC], BF, tag="wsb")
        nc.vector.tensor_scalar(out=wsb[:], in0=wps[:],
                                scalar1=cb_sb[:, n:n + 1], scalar2=None,
                                op0=ADD)
        nc.sync.dma_start(out=wstk_sb[n:n + 1, :], in_=wsb[0:1, :])

        # e' = w * e2, in place. On gpsimd (pure SBUF op), NOT vector: the
        # vector queue is in-order, and these 8 multiplies would otherwise
        # head-of-line-block the previous expert's res += v adds, which gate
        # the chain PSUM pool and stall the PE near the end of the expert
        # loop. The last expert's multiplies are split vector/gpsimd (vector
        # half emitted BEFORE the v(n-1) adds) so every e' tile is ready
        # well before v(7)'s first chain, which reads all 8 of them.
        def emit_mults():
            for m in range(KD):
                if last and m % 2 == 1:
                    nc.vector.tensor_tensor(out=e2_sb[:, m, :], in0=wsb[:],
                                            in1=e2_sb[:, m, :], op=MULT)
                else:
                    nc.gpsimd.tensor_tensor(out=e2_sb[:, m, :], in0=wsb[:],
                                            in1=e2_sb[:, m, :], op=MULT)

        if last:
            emit_mults()
        if pending_v is not None:
            ms = range(KD) if last else range(KD // 2, KD)
            emit_v_phase(*pending_v, ms=ms)
        if not last:
            emit_mults()
        pending_v = (n, wv_sb, e2_sb)

        if n == NE - 1:
            # bvt contribution res += bvt_stack.T @ w_stack (K = NE) as one
            # contiguous block of K=8 matmuls into the now-free pw pool.
            # Placed after expert 6's v-phase so the wstk row-7 copy has
            # landed long before the PE gets here.
            for m in range(KD):
                bp = pw.tile([P, BC], F32, name="wps")
                nc.tensor.matmul(bp[:], bvt_sb[:, _mm(m)], wstk_sb[:],
                                 start=True, stop=True)
                nc.vector.tensor_tensor(out=res_sb[:, m, :],
                                        in0=res_sb[:, m, :], in1=bp[:], op=ADD)

    emit_v_phase(*pending_v)
    t_sb = t_holder[0]

    # ---- head ----
    wl_sb = wexp.tile([P, KD, OUT], BF, tag="wkb")
    nc.sync.dma_start(out=wl_sb[:], in_=io["wl"][:])
    bl_sb = consts.tile([P, 1], F32, tag="bl")
    nc.gpsimd.dma_start(out=bl_sb[:], in_=io["bl"][:])

    # final layer + head, split in four batch quarters so the head ops and
    # output DMAs of earlier quarters overlap the matmuls of later ones, and
    # the post-last-matmul tail is only one quarter's epilogue. mean lives on
    # partitions 0:64 and clipped log_std on 64:128 of ONE tile (engine ops
    # are lane-aligned, so each head half stays on its own partitions), so
    # each quarter needs a single sync-queue DMA. std = exp(log_std) is
    # computed on the host; gpsimd is kept out of the tail because its
    # software DMA queue drains slowly at kernel end.
    H = OUT // 2  # 64
    out_sb = consts.tile([P, BC], F32, tag="out")
    NQ = 4
    HB = BC // NQ
    for h in range(NQ):
        cs = slice(h * HB, (h + 1) * HB)
        po = pchain.tile([P, HB], F32, name="chain")
        for k in range(KD):
            nc.tensor.matmul(po[:], wl_sb[:, k, :], t_sb[:, k, cs],
                             start=(k == 0), stop=(k == KD - 1))
        nc.scalar.activation(out_sb[0:H, cs], po[0:H, :], IDN,
                             bias=bl_sb[0:H, 0:1])
        nc.vector.tensor_scalar(out=out_sb[H:OUT, cs], in0=po[H:OUT, :],
                                scalar1=bl_sb[H:OUT, 0:1], scalar2=LOG_SIG_MIN,
                                op0=ADD, op1=MAX)
        nc.vector.tensor_scalar(out=out_sb[H:OUT, cs], in0=out_sb[H:OUT, cs],
                                scalar1=LOG_SIG_MAX, scalar2=None, op0=MIN)
        nc.sync.dma_start(out=io["out_t"][:, cs], in_=out_sb[:, cs])


def _build_program():
    nc = bacc.Bacc("TRN2", target_bir_lowering=False, debug=False,
                   num_devices=NCORES)
    io = {}

    def din(name, shape, dt):
        io[name] = nc.dram_tensor(name, shape, dt, kind="ExternalInput").ap()

    def dout(name, shape, dt):
        io[name] = nc.dram_tensor(name, shape, dt, kind="ExternalOutput").ap()

    HD = DH // 2
    din("xT", [P, KX, BC], BF)
    din("wb1a", [P, KX, P], BF)
    din("wb1b", [P, KX, HD - P], BF)
    din("wb1c", [P, KX, HD], BF)
    din("wb2m", [KD, P, KD, P], BF)
    din("we1", [NE, 2, P, KD, HD], BF)
    din("we2", [NE, 2, P, KD, HD], BF)
    din("wvt", [NE, 2, P, KD, HD], BF)
    din("wke", [NE, P, KD], F32)
    din("wl", [P, KD, OUT], BF)
    din("bb1", [P, KD], F32)
    din("bb2", [P, KD], F32)
    din("be1", [NE, P, KD], F32)
    din("be2", [NE, P, KD], F32)
    din("bt1", [P, KD], F32)
    din("bl", [P, 1], F32)
    din("cb", [P, NE], F32)
    din("bvt", [NE, DH], BF)
    dout("out_t", [OUT, BC], F32)

    with tile.TileContext(nc) as tc:
        with ExitStack() as ctx:
            _build_kernel(ctx, tc, io)
    nc.compile()
    return nc


_PROGRAM = None


def _get_program():
    global _PROGRAM
    if _PROGRAM is None:
        _PROGRAM = _build_program()
    return _PROGRAM


def _prep_host_inputs(x, task_id, Wb1, bb1, Wb2, bb2, We1, be1, We2, be2,
                      Wv, bv, Wk, bk, Wq, bq, Wt1, bt1, Wl, bl):
    bf = ml_dtypes.bfloat16
    f32 = np.float32
    asf = lambda a: np.asarray(a, dtype=f32)

    tid = int(np.asarray(task_id))
    q = asf(Wq)[tid, tid] + asf(bq)[tid]              # [DK]
    wk_eff = np.einsum("ndk,k->nd", asf(Wk), q)       # [NE, DH]
    c = asf(bk) @ q                                   # [NE]

    # tower fold: Wvt_n = Wv_n @ Wt1, bvt_n = bv_n @ Wt1
    wt1 = asf(Wt1)
    wvt = np.einsum("ndv,vh->ndh", asf(Wv), wt1)      # [NE, DH, DH]
    bvt = asf(bv) @ wt1                               # [NE, DH]

    def wT(w, kt):  # [Din, Dout] -> [128, kt, Dout] bf16
        w = asf(w).astype(bf)
        return np.ascontiguousarray(w.reshape(kt, P, w.shape[1]).transpose(1, 0, 2))

    def bT(b):      # [DH] -> [128, KD] fp32
        return np.ascontiguousarray(asf(b).reshape(KD, P).T)

    HD = DH // 2

    def halves(stack):  # [NE, P, KD, DH] -> [NE, 2, P, KD, HD] contiguous
        return np.ascontiguousarray(
            np.stack([stack[:, :, :, 0:HD], stack[:, :, :, HD:DH]], axis=1))

    wb1T = wT(Wb1, KX)
    wb2T = wT(Wb2, KD)
    shared = {
        "wb1a": np.ascontiguousarray(wb1T[:, :, 0:P]),
        "wb1b": np.ascontiguousarray(wb1T[:, :, P:HD]),
        "wb1c": np.ascontiguousarray(wb1T[:, :, HD:DH]),
        "wb2m": np.ascontiguousarray(
            wb2T.reshape(P, KD, KD, P).transpose(2, 0, 1, 3)),
        "we1": halves(np.stack([wT(np.asarray(We1)[n], KD) for n in range(NE)])),
        "we2": halves(np.stack([wT(np.asarray(We2)[n], KD) for n in range(NE)])),
        "wvt": halves(np.stack([wT(wvt[n], KD) for n in range(NE)])),
        "wke": np.stack([bT(wk_eff[n]) for n in range(NE)]),
        "wl": wT(Wl, KD),
        "bb1": bT(bb1),
        "bb2": bT(bb2),
        "be1": np.stack([bT(np.asarray(be1)[n]) for n in range(NE)]),
        "be2": np.stack([bT(np.asarray(be2)[n]) for n in range(NE)]),
        "bt1": bT(bt1),
        "bl": np.ascontiguousarray(asf(bl).reshape(P, 1)),
        "cb": np.ascontiguousarray(np.broadcast_to(c[None, :], (P, NE)).astype(f32)),
        "bvt": np.ascontiguousarray(bvt.astype(bf)),
    }
    xbf = asf(x).astype(bf)
    in_maps = []
    for ci in range(NCORES):
        xc = xbf[ci * BC:(ci + 1) * BC]               # [BC, OBS]
        xT_h = np.ascontiguousarray(
            xc.T.reshape(KX, P, BC).transpose(1, 0, 2))
        m = dict(shared)
        m["xT"] = xT_h
        in_maps.append(m)
    return in_maps


def kernel(**inputs):
    nc = _get_program()
    in_maps = _prep_host_inputs(**inputs)
    res = run_bass_kernel_spmd(nc, in_maps, core_ids=list(range(NCORES)))
    out = np.concatenate([res.results[i]["out_t"] for i in range(NCORES)],
                         axis=1)                       # [OUT, B]
    mean = np.ascontiguousarray(out[:OUT // 2].T, dtype=np.float32)
    log_std = np.ascontiguousarray(out[OUT // 2:].T, dtype=np.float32)
    std = np.exp(log_std)
    return mean, std, log_std


# revision 33
# speedup vs baseline: 1.0320x; 1.0320x over previous
"""Trainium2 Bass kernel for a mixture-of-experts Gaussian policy network.

Network (reference, all fp32):
  h  = relu(relu(x @ Wb1 + bb1) @ Wb2 + bb2)                    [B, DH]
  e_n = relu(relu(h @ We1_n + be1_n) @ We2_n + be2_n)           per expert n
  v_n = e_n @ Wv_n + bv_n ;  k_n = e_n @ Wk_n + bk_n
  q   = Wq[tid, tid] + bq[tid]
  w_n = k_n . q   (raw, unnormalized)
  res = sum_n w_n * v_n                                          [B, DV]
  t   = relu(res @ Wt1 + bt1) ;  out = t @ Wl + bl               [B, 128]
  mean, log_std = split(out); log_std clipped [-20, 2]; std = exp(log_std)

Strategy: pure data parallelism over the batch (4096 rows -> 512 per core,
8 cores, no collectives). On device everything lives transposed
([feature_partitions, batch_free]) so no transposes are ever needed:
  outT = matmul(lhsT=W[Din,Dout] tile, rhs=inT)   (PE computes lhsT.T @ rhs)

Tower fold: w_n is a per-row SCALAR, so
  res @ Wt1 = sum_n w_n * (e2_n @ (Wv_n @ Wt1))
The host precomputes Wvt_n = Wv_n @ Wt1 (same [DH, DH] shape as Wv_n) and
bvt_n = bv_n @ Wt1, eliminating the entire tower matmul phase on device:
t = relu(sum_n w_n*(e2_n @ Wvt_n) + sum_n w_n*bvt_n + bt1).

Host also folds the task-q vector into Wk (wk_eff = Wk_n @ q, c_n = bk_n . q),
so the router weight w_n comes from a rank-1 lhsT trick: lhsT[k,m] = wk_eff[k]
for all m, which materializes w_n broadcast across all 128 partitions
directly in PSUM. The expert weighting then is one fused DVE op per tile:
  e'_n = (w_psum + c_n) * e2_n        (scalar_tensor_tensor, in place)
res accumulates over experts in SBUF fp32 (PSUM is too small to hold
[DH, B] across the expert loop). The bvt_n contribution is a 9th K=8 matmul
(lhsT = bvt stack, rhs = stack of biased w_n rows). All matmuls bf16
(fp32 accum).

Startup: the first ~10us are DMA-latency-bound while xT/wb1 land, and the
PE HAM clock-gate keeps the array at 1.2 GHz until it has been busy ~3.4us.
Dummy warmup matmuls on a memset tile fill the DMA window so the array is
already at 2.4 GHz when real work starts.
"""

import os
import numpy as np
import ml_dtypes
from contextlib import ExitStack

import concourse.bass as bass
import concourse.tile as tile
from concourse import bacc, mybir
from concourse.bass_utils import run_bass_kernel_spmd

P = 128
NCORES = 8
B = 4096
BC = B // NCORES          # 512 batch rows per core
OBS, DH, NE, DK, DV, TASKS, OUT = 512, 1024, 8, 256, 1024, 10, 128
KX = OBS // P             # 4 k-tiles for the input layer
KD = DH // P              # 8 k-tiles for hidden layers
BF = mybir.dt.bfloat16
F32 = mybir.dt.float32
RELU = mybir.ActivationFunctionType.Relu
EXP = mybir.ActivationFunctionType.Exp
IDN = mybir.ActivationFunctionType.Identity
ADD = mybir.AluOpType.add
MULT = mybir.AluOpType.mult
MAX = mybir.AluOpType.max
MIN = mybir.AluOpType.min

LOG_SIG_MIN, LOG_SIG_MAX = -20.0, 2.0

NWARM = int(os.environ.get("NWARM", "14"))  # warmup matmuls (N=512 each)


def _mm(s):  # m-tile column slice
    return slice(s * P, (s + 1) * P)


def _build_kernel(ctx, tc, io):
    nc = tc.nc
    consts = ctx.enter_context(tc.tile_pool(name="consts", bufs=1))
    wexp = ctx.enter_context(tc.tile_pool(name="wexp", bufs=2))
    eact = ctx.enter_context(tc.tile_pool(name="eact", bufs=2))
    pchain = ctx.enter_context(tc.tile_pool(name="pchain", bufs=6, space="PSUM"))
    pw = ctx.enter_context(tc.tile_pool(name="pw", bufs=2, space="PSUM"))

    # ---- persistent tiles + early DMAs ----
    # Startup is DMA-latency-bound: each queue moves ~65 GB/s early on, and
    # a dispatch costs ~0.7us on the issuing engine. Only sync/scalar/gpsimd
    # can DMA; pieces are spread round-robin in consumption order: base1's
    # m=0 chain needs xT k0..3 plus only wb1[:, k, 0:128]; later m-tiles'
    # columns follow.
    xT_sb = consts.tile([P, KX, BC], BF, tag="xT")
    wb1_sb = wexp.tile([P, KX, DH], BF, tag="w1", bufs=3)
    bb1_sb = consts.tile([P, KD], F32, tag="bb1")
    # warmup source: memset is the vector queue's FIRST instruction (vector
    # cannot DMA, so this costs nothing) so the dummy matmuls can start at
    # ~0.5us, before any DMA lands.
    wsrc_sb = consts.tile([P, BC], BF, tag="wsrc")
    nc.vector.memset(wsrc_sb[:], 0.0)
    # all-ones lhsT for the router's cross-partition-sum matmul
    wones_sb = consts.tile([P, P], BF, tag="wones")
    nc.vector.memset(wones_sb[:], 1.0)
    M0 = P          # columns gating the m=0 chain
    HB2 = BC // 2
    # xT in 5 pieces and wb1's m=0 columns first: the m=0 chain's gate is
    # ~0.2 MB per queue (~4us), not the ~0.5 MB (~11us) of the old halves
    # layout. Consumption order after that: wb1 m1-3 cols, then m4-7 cols.
    # Every piece below is CONTIGUOUS in DRAM (host pre-splits the tensors):
    # strided column-slices of a [P, k, DH] DRAM tensor cost far more
    # descriptor-write time on the issuing engine and transfer slower.
    nc.sync.dma_start(out=xT_sb[:, 0, :], in_=io["xT"][:, 0, :])
    nc.scalar.dma_start(out=xT_sb[:, 1, :], in_=io["xT"][:, 1, :])
    nc.gpsimd.dma_start(out=xT_sb[:, 2, :], in_=io["xT"][:, 2, :])
    nc.sync.dma_start(out=xT_sb[:, 3, 0:HB2], in_=io["xT"][:, 3, 0:HB2])
    nc.scalar.dma_start(out=xT_sb[:, 3, HB2:BC], in_=io["xT"][:, 3, HB2:BC])
    nc.gpsimd.dma_start(out=bb1_sb[:], in_=io["bb1"][:])
    nc.gpsimd.dma_start(out=wb1_sb[:, 0, 0:M0], in_=io["wb1a"][:, 0, :])
    nc.sync.dma_start(out=wb1_sb[:, 1, 0:M0], in_=io["wb1a"][:, 1, :])
    nc.scalar.dma_start(out=wb1_sb[:, 2, 0:M0], in_=io["wb1a"][:, 2, :])
    nc.gpsimd.dma_start(out=wb1_sb[:, 3, 0:M0], in_=io["wb1a"][:, 3, :])
    HD = DH // 2
    nc.sync.dma_start(out=wb1_sb[:, 0, M0:HD], in_=io["wb1b"][:, 0, :])
    nc.scalar.dma_start(out=wb1_sb[:, 1, M0:HD], in_=io["wb1b"][:, 1, :])
    nc.gpsimd.dma_start(out=wb1_sb[:, 2, M0:HD], in_=io["wb1b"][:, 2, :])
    nc.sync.dma_start(out=wb1_sb[:, 3, M0:HD], in_=io["wb1b"][:, 3, :])
    nc.scalar.dma_start(out=wb1_sb[:, 0, HD:DH], in_=io["wb1c"][:, 0, :])
    nc.gpsimd.dma_start(out=wb1_sb[:, 1, HD:DH], in_=io["wb1c"][:, 1, :])
    nc.sync.dma_start(out=wb1_sb[:, 2, HD:DH], in_=io["wb1c"][:, 2, :])
    nc.scalar.dma_start(out=wb1_sb[:, 3, HD:DH], in_=io["wb1c"][:, 3, :])
    # wb2 in per-m-tile column chunks (all k each) so base2's m-chains gate
    # on 0.25 MB apiece instead of the whole 2 MB.
    wb2_sb = wexp.tile([P, KD, DH], BF, tag="w2")
    bb2_sb = consts.tile([P, KD], F32, tag="bb2")
    nc.gpsimd.dma_start(out=bb2_sb[:], in_=io["bb2"][:])
    qs = [nc.sync, nc.scalar, nc.gpsimd]
    for m in range(KD):
        qs[m % 3].dma_start(out=wb2_sb[:, :, _mm(m)], in_=io["wb2m"][m])
    cb_sb = consts.tile([P, NE], F32, tag="cb")
    nc.sync.dma_start(out=cb_sb[:], in_=io["cb"][:])
    bvt_sb = consts.tile([NE, DH], BF, tag="bvt")
    nc.gpsimd.dma_start(out=bvt_sb[:], in_=io["bvt"][:])

    h2_sb = consts.tile([P, KD, BC], BF, tag="h2")
    res_sb = consts.tile([P, KD, BC], F32, tag="res")
    wstk_sb = consts.tile([NE, BC], BF, tag="wstk")

    # All matmul chains draw PSUM tiles from ONE 6-deep ring (6 banks) so a
    # chain reuses a PSUM buffer 6 chains back, not 3: with 3-deep rotation
    # the 4th chain of each phase stalls ~216ns on the WAR wait for the
    # phase's first PSUM reader (activation/add), which lags ~2.5 chains at
    # phase starts. pw keeps the other 2 banks.
    def chain_tile():
        return pchain.tile([P, BC], F32, name="chain")

    # ---- PE warmup ----
    # Fill the DMA-bound startup window with dummy matmuls so the HAM
    # clock-gate releases (1.2 -> 2.4 GHz) before the first real matmul.
    for i in range(NWARM):
        wp = chain_tile()
        nc.tensor.matmul(wp[:], wsrc_sb[:, 0:P], wsrc_sb[:],
                         start=True, stop=True)

    # ---- base MLP ----
    h1_sb = eact.tile([P, KD, BC], BF, tag="e1")
    for m in range(KD):
        ps = chain_tile()
        for k in range(KX):
            nc.tensor.matmul(ps[:], wb1_sb[:, k, _mm(m)], xT_sb[:, k, :],
                             start=(k == 0), stop=(k == KX - 1))
        nc.scalar.activation(h1_sb[:, m, :], ps[:], RELU, bias=bb1_sb[:, m:m + 1])
    for m in range(KD):
        ps = chain_tile()
        for k in range(KD):
            nc.tensor.matmul(ps[:], wb2_sb[:, k, _mm(m)], h1_sb[:, k, :],
                             start=(k == 0), stop=(k == KD - 1))
        nc.scalar.activation(h2_sb[:, m, :], ps[:], RELU, bias=bb2_sb[:, m:m + 1])

    # ---- expert loop ----
    # Software-pipelined: expert n's Wvt/res phase (which depends on DVE
    # e'-mult results) is emitted after expert n+1's MLP matmuls, so the
    # PE never stalls waiting on DVE. The tower is folded into Wvt, so the
    # last expert's res tiles go straight to relu (-> t) and the head.
    bt1_sb = consts.tile([P, KD], F32, tag="bt1")
    t_holder = [None]

    def emit_v_phase(n, wv_sb, e2_sb, ms=range(KD)):
        last = (n == NE - 1)
        if last and t_holder[0] is None:
            # t lives in eact's other "e2" rotation slot (expert n-1's,
            # long dead); relu of m-tile k runs as soon as its final add
            # lands, overlapping the remaining v-phase matmuls.
            t_sb = eact.tile([P, KD, BC], BF, tag="e2")
            t_holder[0] = t_sb
        for m in ms:
            vp = chain_tile()
            for k in range(KD):
                nc.tensor.matmul(vp[:], wv_sb[:, k, _mm(m)], e2_sb[:, k, :],
                                 start=(k == 0), stop=(k == KD - 1))
            if n == 0:
                nc.vector.tensor_copy(res_sb[:, m, :], vp[:])
            elif last and m == KD - 1:
                # last m-tile: add+relu in batch quarters so the head
                # chains' k=7 operands are ready ~1us earlier (each head
                # quarter only needs its own column quarter of t).
                QB = BC // 4
                for q in range(4):
                    cq = slice(q * QB, (q + 1) * QB)
                    nc.vector.tensor_tensor(res_sb[:, m, cq], res_sb[:, m, cq],
                                            vp[:, cq], op=ADD)
                    nc.scalar.activation(t_holder[0][:, m, cq],
                                         res_sb[:, m, cq], RELU,
                                         bias=bt1_sb[:, m:m + 1])
            else:
                nc.vector.tensor_tensor(res_sb[:, m, :], res_sb[:, m, :], vp[:],
                                        op=ADD)
                if last:
                    nc.scalar.activation(t_holder[0][:, m, :], res_sb[:, m, :],
                                         RELU, bias=bt1_sb[:, m:m + 1])

    pending_v = None
    for n in range(NE):
        # weight streams spread across queues (sync/vector/gpsimd) so no
        # single ring serializes the 6.3 MB per expert; scalar is kept free
        # for activations.
        # weights in column halves (contiguous in DRAM via host pre-split) so
        # the first m-chains gate on 1 MB instead of 2 MB (matters for expert
        # 0's ramp right after base2). NO dispatches on scalar: the scalar
        # queue runs the activations, and a ~1us DMA dispatch in front of a
        # group's m0 activation delays it enough that the group's m3 chain
        # stalls on the PSUM-reuse (WAR) wait.
        w1_sb = wexp.tile([P, KD, DH], BF, tag="w1", bufs=3)
        nc.sync.dma_start(out=w1_sb[:, :, 0:HD], in_=io["we1"][n, 0])
        nc.sync.dma_start(out=w1_sb[:, :, HD:DH], in_=io["we1"][n, 1])
        b1_sb = wexp.tile([P, KD], F32, tag="be1")
        nc.gpsimd.dma_start(out=b1_sb[:], in_=io["be1"][n])
        wke_sb = wexp.tile([P, KD], F32, tag="wke")
        nc.gpsimd.dma_start(out=wke_sb[:], in_=io["wke"][n])
        w2_sb = wexp.tile([P, KD, DH], BF, tag="w2")
        nc.gpsimd.dma_start(out=w2_sb[:, :, 0:HD], in_=io["we2"][n, 0])
        nc.gpsimd.dma_start(out=w2_sb[:, :, HD:DH], in_=io["we2"][n, 1])
        b2_sb = wexp.tile([P, KD], F32, tag="be2")
        nc.gpsimd.dma_start(out=b2_sb[:], in_=io["be2"][n])
        wv_sb = wexp.tile([P, KD, DH], BF, tag="wv")
        nc.sync.dma_start(out=wv_sb[:, :, 0:HD], in_=io["wvt"][n, 0])
        nc.sync.dma_start(out=wv_sb[:, :, HD:DH], in_=io["wvt"][n, 1])
        if n == NE - 1:
            nc.gpsimd.dma_start(out=bt1_sb[:], in_=io["bt1"][:])

        e1_sb = eact.tile([P, KD, BC], BF, tag="e1")
        for m in range(KD):
            ps = chain_tile()
            for k in range(KD):
                nc.tensor.matmul(ps[:], w1_sb[:, k, _mm(m)], h2_sb[:, k, :],
                                 start=(k == 0), stop=(k == KD - 1))
            nc.scalar.activation(e1_sb[:, m, :], ps[:], RELU, bias=b1_sb[:, m:m + 1])

        e2_sb = eact.tile([P, KD, BC], BF, tag="e2")
        for m in range(KD):
            ps = chain_tile()
            for k in range(KD):
                nc.tensor.matmul(ps[:], w2_sb[:, k, _mm(m)], e1_sb[:, k, :],
                                 start=(k == 0), stop=(k == KD - 1))
            nc.scalar.activation(e2_sb[:, m, :], ps[:], RELU, bias=b2_sb[:, m:m + 1])

        # router weight w_n = e2_n . wk_n: the contraction is done as 8 DVE
        # multiply-accumulates (per-partition partial dots over the k-tile
        # axis; vector engine has slack) + ONE cross-partition-sum matmul
        # with an all-ones lhsT, which also materializes w broadcast on all
        # 128 partitions. This replaces the old 8-matmul rank-1 chain: -7
        # matmuls of PE time per expert (~12us total).
        racc_sb = eact.tile([P, BC], F32, tag="racc")
        raccb_sb = eact.tile([P, BC], BF, tag="raccb")
        nc.vector.tensor_scalar(out=racc_sb[:], in0=e2_sb[:, 0, :],
                                scalar1=wke_sb[:, 0:1], scalar2=None, op0=MULT)
        for k in range(1, KD - 1):
            nc.vector.scalar_tensor_tensor(out=racc_sb[:], in0=e2_sb[:, k, :],
                                           scalar=wke_sb[:, k:k + 1],
                                           in1=racc_sb[:], op0=MULT, op1=ADD)
        nc.vector.scalar_tensor_tensor(out=raccb_sb[:], in0=e2_sb[:, KD - 1, :],
                                       scalar=wke_sb[:, KD - 1:KD],
                                       in1=racc_sb[:], op0=MULT, op1=ADD)

        last = (n == NE - 1)
        # first half of the previous expert's v-phase runs while the DVE
        # dots drain, so the ones-matmul below never waits on them. For the
        # LAST expert the ones-matmul runs first (eats a ~1.6us dot wait) so
        # wsb and the e' multiplies complete while v(n-1) streams --
        # otherwise v(7)'s chains catch up with the multiplies and stall.
        if pending_v is not None and not last:
            emit_v_phase(*pending_v, ms=range(0, KD // 2))

        wps = pw.tile([P, BC], F32, name="wps")
        nc.tensor.matmul(wps[:], wones_sb[:], raccb_sb[:], start=True, stop=True)
        # biased router weight in SBUF (engine PSUM reads need 32-aligned
        # base partitions, so bias the full tile once, then DMA row 0)
        wsb = eact.tile([P, BC], BF, tag="wsb")
        nc.vector.tensor_scalar(out=wsb[:], in0=wps[:],
                                scalar1=cb_sb[:, n:n + 1], scalar2=None,
                                op0=ADD)
        nc.sync.dma_start(out=wstk_sb[n:n + 1, :], in_=wsb[0:1, :])

        # e' = w * e2, in place. On gpsimd (pure SBUF op), NOT vector: the
        # vector queue is in-order, and these 8 multiplies would otherwise
        # head-of-line-block the previous expert's res += v adds, which gate
        # the chain PSUM pool and stall the PE near the end of the expert
        # loop. The last expert's multiplies are split vector/gpsimd (vector
        # half emitted BEFORE the v(n-1) adds) so every e' tile is ready
        # well before v(7)'s first chain, which reads all 8 of them.
        def emit_mults():
            for m in range(KD):
                if last and m % 2 == 1:
                    nc.vector.tensor_tensor(out=e2_sb[:, m, :], in0=wsb[:],
                                            in1=e2_sb[:, m, :], op=MULT)
                else:
                    nc.gpsimd.tensor_tensor(out=e2_sb[:, m, :], in0=wsb[:],
                                            in1=e2_sb[:, m, :], op=MULT)

        if last:
            emit_mults()
        if pending_v is not None:
            ms = range(KD) if last else range(KD // 2, KD)
            emit_v_phase(*pending_v, ms=ms)
        if not last:
            emit_mults()
        pending_v = (n, wv_sb, e2_sb)

        if n == NE - 1:
            # bvt contribution res += bvt_stack.T @ w_stack (K = NE) as one
            # contiguous block of K=8 matmuls into the now-free pw pool.
            # Placed after expert 6's v-phase so the wstk row-7 copy has
            # landed long before the PE gets here.
            for m in range(KD):
                bp = pw.tile([P, BC], F32, name="wps")
                nc.tensor.matmul(bp[:], bvt_sb[:, _mm(m)], wstk_sb[:],
                                 start=True, stop=True)
                nc.vector.tensor_tensor(out=res_sb[:, m, :],
                                        in0=res_sb[:, m, :], in1=bp[:], op=ADD)

    emit_v_phase(*pending_v)
    t_sb = t_holder[0]

    # ---- head ----
    wl_sb = wexp.tile([P, KD, OUT], BF, tag="wkb")
    nc.sync.dma_start(out=wl_sb[:], in_=io["wl"][:])
    bl_sb = consts.tile([P, 1], F32, tag="bl")
    nc.gpsimd.dma_start(out=bl_sb[:], in_=io["bl"][:])

    # final layer + head, split in four batch quarters so the head ops and
    # output DMAs of earlier quarters overlap the matmuls of later ones, and
    # the post-last-matmul tail is only one quarter's epilogue. mean lives on
    # partitions 0:64 and clipped log_std on 64:128 of ONE tile (engine ops
    # are lane-aligned, so each head half stays on its own partitions), so
    # each quarter needs a single sync-queue DMA. std = exp(log_std) is
    # computed on the host; gpsimd is kept out of the tail because its
    # software DMA queue drains slowly at kernel end.
    H = OUT // 2  # 64
    out_sb = consts.tile([P, BC], F32, tag="out")
    NQ = 4
    HB = BC // NQ
    for h in range(NQ):
        cs = slice(h * HB, (h + 1) * HB)
        po = pchain.tile([P, HB], F32, name="chain")
        for k in range(KD):
            nc.tensor.matmul(po[:], wl_sb[:, k, :], t_sb[:, k, cs],
                             start=(k == 0), stop=(k == KD - 1))
        nc.scalar.activation(out_sb[0:H, cs], po[0:H, :], IDN,
                             bias=bl_sb[0:H, 0:1])
        nc.vector.tensor_scalar(out=out_sb[H:OUT, cs], in0=po[H:OUT, :],
                                scalar1=bl_sb[H:OUT, 0:1], scalar2=LOG_SIG_MIN,
                                op0=ADD, op1=MAX)
        nc.vector.tensor_scalar(out=out_sb[H:OUT, cs], in0=out_sb[H:OUT, cs],
                                scalar1=LOG_SIG_MAX, scalar2=None, op0=MIN)
        nc.sync.dma_start(out=io["out_t"][:, cs], in_=out_sb[:, cs])


def _build_program():
    nc = bacc.Bacc("TRN2", target_bir_lowering=False, debug=False,
                   num_devices=NCORES)
    io = {}

    def din(name, shape, dt):
        io[name] = nc.dram_tensor(name, shape, dt, kind="ExternalInput").ap()

    def dout(name, shape, dt):
        io[name] = nc.dram_tensor(name, shape, dt, kind="ExternalOutput").ap()

    HD = DH // 2
    din("xT", [P, KX, BC], BF)
    din("wb1a", [P, KX, P], BF)
    din("wb1b", [P, KX, HD - P], BF)
    din("wb1c", [P, KX, HD], BF)
    din("wb2m", [KD, P, KD, P], BF)
    din("we1", [NE, 2, P, KD, HD], BF)
    din("we2", [NE, 2, P, KD, HD], BF)
    din("wvt", [NE, 2, P, KD, HD], BF)
    din("wke", [NE, P, KD], F32)
    din("wl", [P, KD, OUT], BF)
    din("bb1", [P, KD], F32)
    din("bb2", [P, KD], F32)
    din("be1", [NE, P, KD], F32)
    din("be2", [NE, P, KD], F32)
    din("bt1", [P, KD], F32)
    din("bl", [P, 1], F32)
    din("cb", [P, NE], F32)
    din("bvt", [NE, DH], BF)
    dout("out_t", [OUT, BC], F32)

    with tile.TileContext(nc) as tc:
        with ExitStack() as ctx:
            _build_kernel(ctx, tc, io)
    nc.compile()
    return nc


_PROGRAM = None


def _get_program():
    global _PROGRAM
    if _PROGRAM is None:
        _PROGRAM = _build_program()
    return _PROGRAM


def _prep_host_inputs(x, task_id, Wb1, bb1, Wb2, bb2, We1, be1, We2, be2,
                      Wv, bv, Wk, bk, Wq, bq, Wt1, bt1, Wl, bl):
    bf = ml_dtypes.bfloat16
    f32 = np.float32
    asf = lambda a: np.asarray(a, dtype=f32)

    tid = int(np.asarray(task_id))
    q = asf(Wq)[tid, tid] + asf(bq)[tid]              # [DK]
    wk_eff = np.einsum("ndk,k->nd", asf(Wk), q)       # [NE, DH]
    c = asf(bk) @ q                                   # [NE]

    # tower fold: Wvt_n = Wv_n @ Wt1, bvt_n = bv_n @ Wt1
    wt1 = asf(Wt1)
    wvt = np.einsum("ndv,vh->ndh", asf(Wv), wt1)      # [NE, DH, DH]
    bvt = asf(bv) @ wt1                               # [NE, DH]

    def wT(w, kt):  # [Din, Dout] -> [128, kt, Dout] bf16
        w = asf(w).astype(bf)
        return np.ascontiguousarray(w.reshape(kt, P, w.shape[1]).transpose(1, 0, 2))

    def bT(b):      # [DH] -> [128, KD] fp32
        return np.ascontiguousarray(asf(b).reshape(KD, P).T)

    HD = DH // 2

    def halves(stack):  # [NE, P, KD, DH] -> [NE, 2, P, KD, HD] contiguous
        return np.ascontiguousarray(
            np.stack([stack[:, :, :, 0:HD], stack[:, :, :, HD:DH]], axis=1))

    wb1T = wT(Wb1, KX)
    wb2T = wT(Wb2, KD)
    shared = {
        "wb1a": np.ascontiguousarray(wb1T[:, :, 0:P]),
        "wb1b": np.ascontiguousarray(wb1T[:, :, P:HD]),
        "wb1c": np.ascontiguousarray(wb1T[:, :, HD:DH]),
        "wb2m": np.ascontiguousarray(
            wb2T.reshape(P, KD, KD, P).transpose(2, 0, 1, 3)),
        "we1": halves(np.stack([wT(np.asarray(We1)[n], KD) for n in range(NE)])),
        "we2": halves(np.stack([wT(np.asarray(We2)[n], KD) for n in range(NE)])),
        "wvt": halves(np.stack([wT(wvt[n], KD) for n in range(NE)])),
        "wke": np.stack([bT(wk_eff[n]) for n in range(NE)]),
        "wl": wT(Wl, KD),
        "bb1": bT(bb1),
        "bb2": bT(bb2),
        "be1": np.stack([bT(np.asarray(be1)[n]) for n in range(NE)]),
        "be2": np.stack([bT(np.asarray(be2)[n]) for n in range(NE)]),
        "bt1": bT(bt1),
        "bl": np.ascontiguousarray(asf(bl).reshape(P, 1)),
        "cb": np.ascontiguousarray(np.broadcast_to(c[None, :], (P, NE)).astype(f32)),
        "bvt": np.ascontiguousarray(bvt.astype(bf)),
    }
    xbf = asf(x).astype(bf)
    in_maps = []
    for ci in range(NCORES):
        xc = xbf[ci * BC:(ci + 1) * BC]               # [BC, OBS]
        xT_h = np.ascontiguousarray(
            xc.T.reshape(KX, P, BC).transpose(1, 0, 2))
        m = dict(shared)
        m["xT"] = xT_h
        in_maps.append(m)
    return in_maps


def kernel(**inputs):
    nc = _get_program()
    in_maps = _prep_host_inputs(**inputs)
    res = run_bass_kernel_spmd(nc, in_maps, core_ids=list(range(NCORES)))
    out = np.concatenate([res.results[i]["out_t"] for i in range(NCORES)],
                         axis=1)                       # [OUT, B]
    mean = np.ascontiguousarray(out[:OUT // 2].T, dtype=np.float32)
    log_std = np.ascontiguousarray(out[OUT // 2:].T, dtype=np.float32)
    std = np.exp(log_std)
    return mean, std, log_std
